# revision 1
# baseline (speedup 1.0000x reference)
"""Trainium2 Bass kernel for the additive-attention problem.

reference math:
    rec[b,h]    = sum_r rnn_state[b,r] * W_rec[h,r]
    scores[t,b] = sum_h tanh(enc[t,b,h] + rec[b,h]) * w_score[h] + b_score + mask[t,b]
    out         = softmax(scores, axis=t)          # (T, B) float32

Sharding: data-parallel over B across 8 cores (4 batch columns per core).
Everything is core-local (softmax is over T), so no collectives.

Per-core pipeline (T=4096, BL=4, H=512) - the big tensor never touches
the PE array (f32 LDWEIGHTS is 4-pass; a transpose-based design was
TensorE-bound at ~267us). Measured: 159us exec; DMA ~94us active
(~340GB/s), VectorE ~135us busy (bottleneck), ScalarE ~130us; first V
op at ~21us (rec chain: bf16 matmuls fed by SWDGE, selector-matmul
broadcast, enc prefetch gated on wrec); ~10us Tile drain tail.
  - DMA enc tile (256 t rows) -> SBUF natural layout (p=t%128, f=(tsub,b,h)),
    both tsub-half dma_starts issued from the idle Sync engine (issuing
    from ACT couples DMA to tanh work; HWDGE queue fan-out is by shape,
    not issuing engine); first 5 tiles gated on the wrec load so the
    startup burst cannot starve the rec chain
  - VectorE adds broadcast rec (one (128,4096) f32 TT per tile, 1x)
  - ScalarE tanh f32 -> bf16
  - VectorE multiply by broadcast w_score (bf16 2x mode)
  - reduction over h split: 5 of 8 (tsub,b) units per tile via ScalarE
    activation-Copy accum_out, the rest via VectorE bf16 add-tree (2 levels,
    2x mode) + tensor_reduce -> scores_all (128, (i,tsub,b)) f32
  - mask added with one 128x128 tensor_tensor
  - ScalarE exp; one PE transpose -> (p=(i,tsub,b), f=t%128); VectorE row
    sums; block-mask matmul broadcasts per-b totals; reciprocal;
    tensor_scalar_mul; DMA out as (BL, T) contiguous 512B runs.
b_score cancels in softmax and is ignored.  No max-subtraction needed:
|scores| <= ||w_score||_1 + o(1) <~ 25, safely inside f32 exp range.
bf16 is used only after tanh (values in [-1,1]); observed rel err ~1e-3.
"""

import numpy as np

T, B, H, R = 4096, 32, 512, 512
NCORES = 8
BL = B // NCORES          # 4 local batch columns
TT = 256                  # t rows per tile
NTILES = T // TT          # 16
TSUB = TT // 128          # 2
HC = H // 128             # 4 h-chunks (rec matmul only)

_GRAPH = None


def _build_graph():
    import concourse.bass as bass
    import concourse.tile as tile
    from concourse import bacc, mybir
    from concourse.masks import make_identity

    f32 = mybir.dt.float32
    bf16 = mybir.dt.bfloat16
    nc = bacc.Bacc()

    enc = nc.declare_dram_parameter("enc", [T, BL, H], f32, isOutput=False)
    maskd = nc.declare_dram_parameter("maskd", [T, BL], f32, isOutput=False)
    rnnT = nc.declare_dram_parameter("rnnT", [R, BL], bf16, isOutput=False)
    wrecT = nc.declare_dram_parameter("wrecT", [R, H], bf16, isOutput=False)
    wscb = nc.declare_dram_parameter("wscb", [128, H], f32, isOutput=False)
    m4d = nc.declare_dram_parameter("m4", [128, 128], f32, isOutput=False)
    out = nc.declare_dram_parameter("out", [BL, T], f32, isOutput=True)


    with tile.TileContext(nc) as tc:
        with (
            tc.tile_pool(name="singles", bufs=1) as singles,
            tc.tile_pool(name="xpool", bufs=5) as xpool,
            tc.tile_pool(name="ypool", bufs=3) as ypool,
            tc.tile_pool(name="scratch", bufs=2) as scratch,
            tc.tile_pool(name="spool", bufs=2, space="PSUM") as spool,
        ):
            # ---------- constants / setup ----------
            # rec-chain inputs first, on SWDGE (uncontended by enc prefetch)
            rnn_sb = singles.tile([128, HC, BL], bf16)
            nc.gpsimd.dma_start(
                out=rnn_sb[:], in_=rnnT.rearrange("(rc p) b -> p rc b", p=128)
            )
            wrec_sb = singles.tile([128, HC, H], bf16)
            wrec_dma = nc.gpsimd.dma_start(
                out=wrec_sb[:], in_=wrecT.rearrange("(rc p) h -> p rc h", p=128)
            )
            ident = singles.tile([128, 128], f32)
            make_identity(nc, ident[:])

            m4 = singles.tile([128, 128], f32)
            nc.sync.dma_start(out=m4[:], in_=m4d[:])

            # w_score broadcast to all partitions, converted to bf16
            w_f32 = singles.tile([128, H], f32)
            nc.scalar.dma_start(out=w_f32[:], in_=wscb[:])
            w_bf = singles.tile([128, H], bf16)
            nc.vector.tensor_copy(out=w_bf[:], in_=w_f32[:])
            w8_bf = singles.tile([128, TSUB, BL, H], bf16)
            for ts in range(TSUB):
                for b in range(BL):
                    nc.vector.tensor_copy(out=w8_bf[:, ts, b, :], in_=w_f32[:])

            # mask in natural layout: (p=t%128, f=(i*tsub, b))
            mask_sb = singles.tile([128, NTILES * TSUB, BL], f32)
            nc.sync.dma_start(
                out=mask_sb[:], in_=maskd.rearrange("(its p) b -> p its b", p=128)
            )

            # rec = rnn @ W_rec.T   via 4 accumulating matmuls over r-chunks
            rec_ps = spool.tile([BL, H], f32, tag="scores")
            for rc in range(HC):
                nc.tensor.matmul(
                    rec_ps[:],
                    lhsT=rnn_sb[:, rc, :],
                    rhs=wrec_sb[:, rc, :],
                    start=(rc == 0),
                    stop=(rc == HC - 1),
                )
            rec_sb4 = singles.tile([BL, H], bf16)
            nc.vector.tensor_copy(out=rec_sb4[:], in_=rec_ps[:])
            # broadcast (BL,H) -> (128, BL, H): one-hot row-selector matmuls
            sel = singles.tile([BL, BL, 128], f32)
            nc.gpsimd.memset(sel[:], 0.0)
            nc.gpsimd.affine_select(
                out=sel[:],
                in_=sel[:],
                compare_op=mybir.AluOpType.not_equal,
                fill=1.0,
                base=0,
                # sel[k, b, m] = (k - b) != 0 ? 0.0 : 1.0
                pattern=[[-1, BL], [0, 128]],
                channel_multiplier=1,
            )
            sel_bf = singles.tile([BL, BL, 128], bf16)
            nc.vector.tensor_copy(out=sel_bf[:], in_=sel[:])
            rec_rep2 = singles.tile([128, TSUB, BL, H], f32)
            for b in range(BL):
                rb_ps = spool.tile([128, H], f32, tag="scores")
                nc.tensor.matmul(
                    rb_ps[:],
                    lhsT=sel_bf[:, b, :],
                    rhs=rec_sb4[:],
                    start=True,
                    stop=True,
                )
                nc.vector.tensor_copy(out=rec_rep2[:, 0, b, :], in_=rb_ps[:])
            nc.vector.tensor_copy(out=rec_rep2[:, 1], in_=rec_rep2[:, 0])

            scores_all = singles.tile([128, NTILES * TSUB * BL], f32)  # (128,128)

            # ---------- main loop over t tiles ----------
            import os as _os
            encv = enc.rearrange("(i ts p) b h -> i p ts (b h)", p=128, ts=TSUB)
            for i in range(NTILES):
                X = xpool.tile([128, TSUB, BL, H], f32)
                ev = encv[i].rearrange("p ts c -> p ts c")
                d0 = nc.sync.dma_start(out=X[:, 0], in_=ev[:, 0])
                d1 = nc.sync.dma_start(out=X[:, 1], in_=ev[:, 1])
                if i < 5:
                    # keep the startup prefetch burst from starving the
                    # small wrec load on the shared SDMA engines
                    from concourse.tile_rust import add_dep_helper
                    add_dep_helper(
                        d0.ins, wrec_dma.ins, sync=True,
                        reason="gate enc prefetch on wrec",
                    )
                    add_dep_helper(
                        d1.ins, wrec_dma.ins, sync=True,
                        reason="gate enc prefetch on wrec",
                    )
                nc.vector.tensor_add(out=X[:], in0=X[:], in1=rec_rep2[:])

                Y = ypool.tile([128, TSUB, BL, H], bf16)
                nc.scalar.activation(
                    out=Y[:],
                    in_=X[:],
                    func=mybir.ActivationFunctionType.Tanh,
                )

                # prod = Y * w  (bf16 2x); reduction split between S and V:
                # first K_SRED of the 8 (ts,b) units via ScalarE activation
                # accum, the rest via V 2-level add tree + reduce.
                nsred = int(_os.environ.get("K_SRED", "5"))
                prod = scratch.tile([128, TSUB, BL, H], bf16, tag="prod")
                nc.vector.tensor_mul(out=prod[:], in0=Y[:], in1=w8_bf[:])
                base = i * TSUB * BL
                units = [(ts, b) for ts in range(TSUB) for b in range(BL)]
                for u, (ts, b) in enumerate(units[:nsred]):
                    dummy = scratch.tile([128, H], bf16, tag="sdump")
                    nc.scalar.activation(
                        out=dummy[:],
                        in_=prod[:, ts, b, :],
                        func=mybir.ActivationFunctionType.Copy,
                        accum_out=scores_all[:, base + u : base + u + 1],
                    )
                if nsred < TSUB * BL:
                    # V path over the remaining units (contiguous tail)
                    rest = prod[:].rearrange("p ts b h -> p (ts b) h")[
                        :, nsred:, :
                    ]
                    h2, h4 = H // 2, H // 4
                    nc.vector.tensor_add(
                        out=rest[:, :, :h2],
                        in0=rest[:, :, :h2],
                        in1=rest[:, :, h2:],
                    )
                    nc.vector.tensor_add(
                        out=rest[:, :, :h4],
                        in0=rest[:, :, :h4],
                        in1=rest[:, :, h4:h2],
                    )
                    nc.vector.tensor_reduce(
                        out=scores_all[:, base + nsred : base + TSUB * BL],
                        in_=rest[:, :, :h4],
                        axis=mybir.AxisListType.X,
                        op=mybir.AluOpType.add,
                    )

            # ---------- mask, exp, softmax normalization, output ----------
            nc.vector.tensor_add(
                out=scores_all[:],
                in0=scores_all[:],
                in1=mask_sb[:].rearrange("p a b -> p (a b)"),
            )
            E = singles.tile([128, 128], f32)
            nc.scalar.activation(
                out=E[:], in_=scores_all[:],
                func=mybir.ActivationFunctionType.Exp,
            )
            # transpose: (p=t%128, f=(i,ts,b)) -> (p=(i,ts,b), f=t%128)
            attT = spool.tile([128, 128], f32, tag="scores")
            nc.tensor.transpose(out=attT[:], in_=E[:], identity=ident[:])
            row_sums = singles.tile([128, 1], f32)
            nc.vector.tensor_reduce(
                out=row_sums[:], in_=attT[:], axis=mybir.AxisListType.X,
                op=mybir.AluOpType.add,
            )
            denom = spool.tile([128, 1], f32, tag="scores")
            nc.tensor.matmul(
                denom[:], lhsT=m4[:], rhs=row_sums[:], start=True, stop=True
            )
            recip = singles.tile([128, 1], f32)
            nc.vector.reciprocal(out=recip[:], in_=denom[:])
            att_out = singles.tile([128, 128], f32)
            nc.vector.tensor_scalar_mul(
                out=att_out[:], in0=attT[:], scalar1=recip[:]
            )
            # partition p = (i, ts, b) holds 128 contiguous t values for col b
            nc.sync.dma_start(
                out=out.rearrange("b (its tp) -> its b tp", tp=128),
                in_=att_out[:],
            )

    nc.compile()
    return nc


def _get_graph():
    global _GRAPH
    if _GRAPH is None:
        _GRAPH = _build_graph()
    return _GRAPH


def make_in_maps(enc, mask, rnn_state, W_rec, w_score):
    import ml_dtypes

    enc = np.ascontiguousarray(enc, dtype=np.float32)
    wrecT = np.ascontiguousarray(
        W_rec.T.astype(np.float32).astype(ml_dtypes.bfloat16)
    )
    wscb = np.ascontiguousarray(
        np.broadcast_to(w_score.astype(np.float32)[None, :], (128, H))
    )
    m4 = (np.arange(128)[:, None] % BL == np.arange(128)[None, :] % BL).astype(
        np.float32
    )
    in_maps = []
    for c in range(NCORES):
        sl = slice(c * BL, (c + 1) * BL)
        in_maps.append(
            {
                "enc": np.ascontiguousarray(enc[:, sl, :]),
                "maskd": np.ascontiguousarray(mask[:, sl].astype(np.float32)),
                "rnnT": np.ascontiguousarray(
                    rnn_state[sl].T.astype(np.float32).astype(ml_dtypes.bfloat16)
                ),
                "wrecT": wrecT,
                "wscb": wscb,
                "m4": m4,
            }
        )
    return in_maps


def kernel(
    encoded_contribution,
    mask,
    rnn_state,
    prev_att_weights,
    W_rec,
    w_score,
    b_score,
):
    from concourse.bass_utils import run_bass_kernel_spmd

    nc = _get_graph()
    in_maps = make_in_maps(
        np.asarray(encoded_contribution),
        np.asarray(mask),
        np.asarray(rnn_state),
        np.asarray(W_rec),
        np.asarray(w_score),
    )
    res = run_bass_kernel_spmd(nc, in_maps, list(range(NCORES)))
    outs = [np.asarray(res.results[c]["out"]) for c in range(NCORES)]
    return np.concatenate([o.T for o in outs], axis=1).astype(np.float32)



# revision 2
# speedup vs baseline: 1.8342x; 1.8342x over previous
"""Trainium2 Bass kernel for the additive-attention problem.

reference math:
    rec[b,h]    = sum_r rnn_state[b,r] * W_rec[h,r]
    scores[t,b] = sum_h tanh(enc[t,b,h] + rec[b,h]) * w_score[h] + b_score + mask[t,b]
    out         = softmax(scores, axis=t)          # (T, B) float32

Sharding: data-parallel over B across 8 cores (BL=4 batch columns per core).
Softmax is over T (core-local), so no collectives.

Design (h-major layout, enc pre-staged on host as bf16):
  - host stages enc as [tile, p=h%128, hc=h//128, b, t] bf16 -> each t-tile
    is one contiguous 1MB DMA; total enc traffic halves vs f32 (16MB/core,
    ~47us at ~350GB/s).
  - rec computed on device: 16 small bf16 matmuls (W_rec^T chunks as lhsT)
    -> rec in (p=h%128, hc, b) f32.  In this layout rec is a per-partition
    scalar for each (hc,b) slice, so the broadcast add is VectorE
    tensor_scalar_add in 4x bf16 mode (16 ops/tile, ~2us/tile).
  - tanh: ONE ScalarE activation per tile over the whole (128, 4096) tile
    (bf16 in/out), ~3.6us/tile; the 224cyc/instr overhead is paid 16x only.
  - score reduction over h moves to the TensorEngine: for each 128
    consecutive t (fixed b), lhsT = Y slice (p=h, m=t) stationary, rhs =
    w_score chunk (128,1) bf16 -> out column (p=t%128) of a persistent
    PSUM scores tile, accumulated over the 4 h-chunks (start/stop).
    512 LDW+MM pairs total; FWL (auto for 128-col bf16 weights) keeps the
    weight-load port at ~2x.
  - epilogue identical to the t-major baseline: V mask add (psum src),
    ScalarE exp, PE transpose, V row sums, block-mask matmul to broadcast
    per-b totals, reciprocal, scale, output DMA as (BL,T) 512B runs.
b_score cancels in softmax and is ignored.  No max-subtraction needed:
|scores| <= ||w_score||_1 <~ 25, safely inside f32 exp range.
"""

import numpy as np

T, B, H, R = 4096, 32, 512, 512
NCORES = 8
BL = B // NCORES          # 4 local batch columns
TT = 256                  # t rows per tile
NT = T // TT              # 16
TS = TT // 128            # 2
HC = H // 128             # 4 h-chunks
RC = R // 128             # 4 r-chunks

_GRAPH = None


def _build_graph():
    import concourse.bass as bass
    import concourse.tile as tile
    from concourse import bacc, mybir
    from concourse.masks import make_identity

    f32 = mybir.dt.float32
    bf16 = mybir.dt.bfloat16
    nc = bacc.Bacc()

    encT = nc.declare_dram_parameter(
        "encT", [NT, 128, HC, BL, TT], bf16, isOutput=False
    )
    maskd = nc.declare_dram_parameter("maskd", [T, BL], f32, isOutput=False)
    rnnT = nc.declare_dram_parameter("rnnT", [RC, 128, BL], bf16, isOutput=False)
    wrecT = nc.declare_dram_parameter("wrecT", [RC, 128, H], bf16, isOutput=False)
    wT = nc.declare_dram_parameter("wT", [128, HC], bf16, isOutput=False)
    m4d = nc.declare_dram_parameter("m4", [128, 128], f32, isOutput=False)
    out = nc.declare_dram_parameter("out", [BL, T], f32, isOutput=True)

    with tile.TileContext(nc) as tc:
        with (
            tc.tile_pool(name="singles", bufs=1) as singles,
            tc.tile_pool(name="xpool", bufs=4) as xpool,
            tc.tile_pool(name="ypool", bufs=3) as ypool,
            tc.tile_pool(name="spsum", bufs=1, space="PSUM") as spsum,
            tc.tile_pool(name="epsum", bufs=2, space="PSUM") as epsum,
        ):
            # ---------- small inputs first (FIFO per HWDGE ring) ----------
            rnn_sb = singles.tile([128, RC, BL], bf16)
            nc.sync.dma_start(out=rnn_sb[:], in_=rnnT.rearrange("r p b -> p r b"))
            wrec_sb = singles.tile([128, RC, H], bf16)
            nc.sync.dma_start(out=wrec_sb[:], in_=wrecT.rearrange("r p h -> p r h"))
            w_sb = singles.tile([128, HC], bf16)
            nc.sync.dma_start(out=w_sb[:], in_=wT[:])
            mask_sb = singles.tile([128, NT * TS, BL], f32)
            nc.sync.dma_start(
                out=mask_sb[:], in_=maskd.rearrange("(a p) b -> p a b", p=128)
            )
            m4 = singles.tile([128, 128], f32)
            nc.sync.dma_start(out=m4[:], in_=m4d[:])
            ident = singles.tile([128, 128], f32)
            make_identity(nc, ident[:])

            # ---------- rec[h, b] = sum_r W_rec[h,r] rnn[b,r] ----------
            rec_ps = epsum.tile([128, HC, BL], f32, tag="epi")
            for hc in range(HC):
                for rc in range(RC):
                    nc.tensor.matmul(
                        rec_ps[:, hc, :],
                        lhsT=wrec_sb[:, rc, hc * 128 : (hc + 1) * 128],
                        rhs=rnn_sb[:, rc, :],
                        start=(rc == 0),
                        stop=(rc == RC - 1),
                    )
            rec_sb = singles.tile([128, HC, BL], f32)
            nc.vector.tensor_copy(out=rec_sb[:], in_=rec_ps[:])

            # persistent scores accumulator: (p=t%128, f=(i,ts,b))
            scores_ps = spsum.tile([128, NT * TS * BL], f32)

            encv = encT.rearrange("i p c b t -> i p c b t")

            # ---------- main loop over t tiles ----------
            for i in range(NT):
                X = xpool.tile([128, HC, BL, TT], bf16)
                nc.sync.dma_start(out=X[:], in_=encv[i])
                for hc in range(HC):
                    for b in range(BL):
                        nc.vector.tensor_scalar_add(
                            out=X[:, hc, b, :],
                            in0=X[:, hc, b, :],
                            scalar1=rec_sb[:, hc, b : b + 1],
                        )
                Y = ypool.tile([128, HC, BL, TT], bf16)
                nc.scalar.activation(
                    out=Y[:],
                    in_=X[:],
                    func=mybir.ActivationFunctionType.Tanh,
                )
                for ts in range(TS):
                    for b in range(BL):
                        c = (i * TS + ts) * BL + b
                        for hc in range(HC):
                            nc.tensor.matmul(
                                scores_ps[:, c : c + 1],
                                lhsT=Y[:, hc, b, ts * 128 : (ts + 1) * 128],
                                rhs=w_sb[:, hc : hc + 1],
                                start=(hc == 0),
                                stop=(hc == HC - 1),
                            )

            # ---------- mask, exp, softmax normalization, output ----------
            sc = singles.tile([128, 128], f32)
            nc.vector.tensor_add(
                out=sc[:],
                in0=scores_ps[:],
                in1=mask_sb[:].rearrange("p a b -> p (a b)"),
            )
            E = singles.tile([128, 128], f32)
            nc.scalar.activation(
                out=E[:], in_=sc[:], func=mybir.ActivationFunctionType.Exp
            )
            # transpose: (p=t%128, f=(i,ts,b)) -> (p=(i,ts,b), f=t%128)
            attT = epsum.tile([128, 128], f32, tag="epi")
            nc.tensor.transpose(out=attT[:], in_=E[:], identity=ident[:])
            row_sums = singles.tile([128, 1], f32)
            nc.vector.tensor_reduce(
                out=row_sums[:], in_=attT[:], axis=mybir.AxisListType.X,
                op=mybir.AluOpType.add,
            )
            denom = epsum.tile([128, 1], f32, tag="epi")
            nc.tensor.matmul(
                denom[:], lhsT=m4[:], rhs=row_sums[:], start=True, stop=True
            )
            recip = singles.tile([128, 1], f32)
            nc.vector.reciprocal(out=recip[:], in_=denom[:])
            att_out = singles.tile([128, 128], f32)
            nc.vector.tensor_scalar_mul(
                out=att_out[:], in0=attT[:], scalar1=recip[:]
            )
            # partition p = (i, ts, b) holds 128 contiguous t values for col b
            nc.sync.dma_start(
                out=out.rearrange("b (a tp) -> a b tp", tp=128),
                in_=att_out[:],
            )

    nc.compile()
    return nc


def _get_graph():
    global _GRAPH
    if _GRAPH is None:
        _GRAPH = _build_graph()
    return _GRAPH


def make_in_maps(enc, mask, rnn_state, W_rec, w_score):
    import ml_dtypes

    bf = ml_dtypes.bfloat16
    enc_bf = np.asarray(enc, dtype=np.float32).astype(bf)
    # [i, t, core, b, hc, p] view -> per-core [i, p, hc, b, t]
    e6 = enc_bf.reshape(NT, TT, NCORES, BL, HC, 128).transpose(2, 0, 5, 4, 3, 1)
    wrecT = np.ascontiguousarray(
        W_rec.T.astype(np.float32).astype(bf).reshape(RC, 128, H)
    )
    wTh = np.ascontiguousarray(
        w_score.astype(np.float32).astype(bf).reshape(HC, 128).T
    )
    m4 = (np.arange(128)[:, None] % BL == np.arange(128)[None, :] % BL).astype(
        np.float32
    )
    mask = np.asarray(mask, dtype=np.float32)
    rnn = np.asarray(rnn_state, dtype=np.float32)
    in_maps = []
    for c in range(NCORES):
        sl = slice(c * BL, (c + 1) * BL)
        in_maps.append(
            {
                "encT": np.ascontiguousarray(e6[c]),
                "maskd": np.ascontiguousarray(mask[:, sl]),
                "rnnT": np.ascontiguousarray(
                    rnn[sl].T.astype(bf).reshape(RC, 128, BL)
                ),
                "wrecT": wrecT,
                "wT": wTh,
                "m4": m4,
            }
        )
    return in_maps


def kernel(
    encoded_contribution,
    mask,
    rnn_state,
    prev_att_weights,
    W_rec,
    w_score,
    b_score,
):
    from concourse.bass_utils import run_bass_kernel_spmd

    nc = _get_graph()
    in_maps = make_in_maps(
        np.asarray(encoded_contribution),
        np.asarray(mask),
        np.asarray(rnn_state),
        np.asarray(W_rec),
        np.asarray(w_score),
    )
    res = run_bass_kernel_spmd(nc, in_maps, list(range(NCORES)))
    outs = [np.asarray(res.results[c]["out"]) for c in range(NCORES)]
    return np.concatenate([o.T for o in outs], axis=1).astype(np.float32)


# revision 3
# speedup vs baseline: 2.0438x; 1.1143x over previous
"""Trainium2 Bass kernel for the additive-attention problem.

reference math:
    rec[b,h]    = sum_r rnn_state[b,r] * W_rec[h,r]
    scores[t,b] = sum_h tanh(enc[t,b,h] + rec[b,h]) * w_score[h] + b_score + mask[t,b]
    out         = softmax(scores, axis=t)          # (T, B) float32

Sharding: data-parallel over B across 8 cores (BL=4 batch columns per core).
Softmax is over T (core-local), so no collectives.

Design (h-major layout, enc pre-staged on host as bf16, v3):
  - host stages enc as [granule=256 t-rows][p=h%128][hc][b][t] bf16; tiles
    are 1-2 granules (schedule 256,256,512x7) -> 1MB DMAs, small first tiles
    for fast pipeline fill, few big tanh instrs in steady state (ScalarE
    ACTIVATE pays ~354ns fixed per instr + ~0.98ns/elem).
  - rec computed on device (16 small bf16 matmuls); in h-major layout rec is
    a per-partition scalar per (hc,b) slice -> VectorE tensor_scalar_add.
  - tanh: ONE ScalarE activation per tile (bf16), the kernel bottleneck
    (~64us of ACTIVATE); activation table pre-warmed with a dummy tanh so
    the ~1.5us ACT_TABLE_LOAD overlaps the first DMAs.
  - mask is pre-added INTO the PSUM scores tile by an identity-matmul before
    the main loop (start=True); all score matmuls then accumulate onto it
    (start=False, per-element has_written semantics).
  - score reduction over h on TensorE: per 128 consecutive t (fixed b),
    lhsT = tanh-slice (p=h, m=t) stationary, rhs = w chunk (128,1) bf16,
    accumulated over 4 h-chunks into one column of the persistent PSUM
    scores tile (p=t%128, f=(t//128, b)).
  - epilogue: ScalarE exp (PSUM src) -> one matmul with rhs=[I|ones] doing
    transpose AND row sums together -> m4 block-mask matmul broadcasts
    per-b totals -> reciprocal -> scale -> output DMA as (BL,T) 512B runs.
b_score cancels in softmax and is ignored.  No max-subtraction needed:
|scores| <= ||w_score||_1 <~ 25, safely inside f32 exp range.
"""

import numpy as np

T, B, H, R = 4096, 32, 512, 512
NCORES = 8
BL = B // NCORES          # 4 local batch columns
GT = 256                  # granule t rows (host staging unit)
NG = T // GT              # 16 granules
HC = H // 128             # 4 h-chunks
RC = R // 128             # 4 r-chunks
# tiles in granules: small first tiles for pipeline fill, 512-row steady
TSCHED = [1, 1, 2, 2, 2, 2, 2, 2, 2]
assert sum(TSCHED) == NG

_GRAPH = None


def _build_graph():
    import concourse.bass as bass
    import concourse.tile as tile
    from concourse import bacc, mybir
    from concourse.masks import make_identity

    f32 = mybir.dt.float32
    bf16 = mybir.dt.bfloat16
    nc = bacc.Bacc()

    encT = nc.declare_dram_parameter(
        "encT", [NG, 128, HC, BL, GT], bf16, isOutput=False
    )
    maskd = nc.declare_dram_parameter("maskd", [T, BL], f32, isOutput=False)
    rnnT = nc.declare_dram_parameter("rnnT", [RC, 128, BL], bf16, isOutput=False)
    wrecT = nc.declare_dram_parameter("wrecT", [RC, 128, H], bf16, isOutput=False)
    wT = nc.declare_dram_parameter("wT", [128, HC], bf16, isOutput=False)
    m4d = nc.declare_dram_parameter("m4", [128, 128], f32, isOutput=False)
    out = nc.declare_dram_parameter("out", [BL, T], f32, isOutput=True)

    with tile.TileContext(nc) as tc:
        with (
            tc.tile_pool(name="singles", bufs=1) as singles,
            tc.tile_pool(name="xpool", bufs=3) as xpool,
            tc.tile_pool(name="ypool", bufs=2) as ypool,
            tc.tile_pool(name="spsum", bufs=1, space="PSUM") as spsum,
            tc.tile_pool(name="epsum", bufs=2, space="PSUM") as epsum,
        ):
            # ---------- enc granule DMAs on the sync HWDGE ring ----------
            # (issued first so the SDMA engines start on tile 0 immediately;
            # small inputs go on the gpsimd SWDGE ring in parallel)
            encv = encT.rearrange("g p c b t -> g p c b t")

            # identity+ones for the combined transpose/row-sum matmul, and
            # an early dummy tanh to pull ACT_TABLE_LOAD off the critical path
            idext = singles.tile([128, 129], f32)
            make_identity(nc, idext[:, 0:128])
            nc.gpsimd.memset(idext[:, 128:129], 1.0)
            warm = singles.tile([128, 1], f32)
            nc.scalar.activation(
                out=warm[:], in_=idext[:, 0:1],
                func=mybir.ActivationFunctionType.Tanh,
            )

            rnn_sb = singles.tile([128, RC, BL], bf16)
            nc.gpsimd.dma_start(out=rnn_sb[:], in_=rnnT.rearrange("r p b -> p r b"))
            wrec_sb = singles.tile([128, RC, H], bf16)
            nc.gpsimd.dma_start(
                out=wrec_sb[:], in_=wrecT.rearrange("r p h -> p r h")
            )
            w_sb = singles.tile([128, HC], bf16)
            nc.gpsimd.dma_start(out=w_sb[:], in_=wT[:])
            mask_sb = singles.tile([128, NG * (GT // 128), BL], f32)
            nc.gpsimd.dma_start(
                out=mask_sb[:], in_=maskd.rearrange("(a p) b -> p a b", p=128)
            )
            m4 = singles.tile([128, 128], f32)
            nc.gpsimd.dma_start(out=m4[:], in_=m4d[:])

            # ---------- rec[h, b] = sum_r W_rec[h,r] rnn[b,r] ----------
            rec_ps = epsum.tile([128, HC, BL], f32, tag="epi")
            for hc in range(HC):
                for rc in range(RC):
                    nc.tensor.matmul(
                        rec_ps[:, hc, :],
                        lhsT=wrec_sb[:, rc, hc * 128 : (hc + 1) * 128],
                        rhs=rnn_sb[:, rc, :],
                        start=(rc == 0),
                        stop=(rc == RC - 1),
                    )
            rec_sb = singles.tile([128, HC, BL], f32)
            nc.vector.tensor_copy(out=rec_sb[:], in_=rec_ps[:])

            # persistent scores accumulator: (p=t%128, f=(t//128, b));
            # seeded with the additive mask (identity matmul, start=True) so
            # every score matmul just accumulates (start=False).
            scores_ps = spsum.tile([128, NG * (GT // 128) * BL], f32)
            nc.tensor.matmul(
                scores_ps[:],
                lhsT=idext[:, 0:128],
                rhs=mask_sb[:].rearrange("p a b -> p (a b)"),
                start=True,
                stop=False,
                skip_group_check=True,
            )

            # ---------- main loop over t tiles ----------
            g0 = 0
            for ng in TSCHED:
                X = xpool.tile([128, ng, HC, BL, GT], bf16)
                for g in range(ng):
                    nc.sync.dma_start(out=X[:, g], in_=encv[g0 + g])
                for hc in range(HC):
                    for b in range(BL):
                        nc.vector.tensor_scalar_add(
                            out=X[:, :, hc, b, :],
                            in0=X[:, :, hc, b, :],
                            scalar1=rec_sb[:, hc, b : b + 1],
                        )
                Y = ypool.tile([128, ng, HC, BL, GT], bf16)
                nc.scalar.activation(
                    out=Y[:],
                    in_=X[:],
                    func=mybir.ActivationFunctionType.Tanh,
                )
                for g in range(ng):
                    for ts in range(GT // 128):
                        a = (g0 + g) * (GT // 128) + ts
                        for b in range(BL):
                            c = a * BL + b
                            for hc in range(HC):
                                nc.tensor.matmul(
                                    scores_ps[:, c : c + 1],
                                    lhsT=Y[:, g, hc, b, ts * 128 : (ts + 1) * 128],
                                    rhs=w_sb[:, hc : hc + 1],
                                    start=False,
                                    stop=(hc == HC - 1),
                                    skip_group_check=True,
                                )
                g0 += ng

            # ---------- exp, transpose+row-sums, normalize, output ----------
            E = singles.tile([128, 128], f32)
            nc.scalar.activation(
                out=E[:], in_=scores_ps[:], func=mybir.ActivationFunctionType.Exp
            )
            # one matmul: cols 0..127 = E^T (p=(a,b), f=t%128), col 128 = row sums
            attx = epsum.tile([128, 129], f32, tag="epi")
            nc.tensor.matmul(
                attx[:], lhsT=E[:], rhs=idext[:], start=True, stop=True
            )
            rs_sb = singles.tile([128, 1], f32)
            nc.vector.tensor_copy(out=rs_sb[:], in_=attx[:, 128:129])
            denom = epsum.tile([128, 1], f32, tag="epi")
            nc.tensor.matmul(
                denom[:], lhsT=m4[:], rhs=rs_sb[:], start=True, stop=True
            )
            recip = singles.tile([128, 1], f32)
            nc.vector.reciprocal(out=recip[:], in_=denom[:])
            att_out = singles.tile([128, 128], f32)
            nc.vector.tensor_scalar_mul(
                out=att_out[:], in0=attx[:, 0:128], scalar1=recip[:]
            )
            # partition p = (a=t//128, b) holds 128 contiguous t values for col b
            nc.sync.dma_start(
                out=out.rearrange("b (a tp) -> a b tp", tp=128),
                in_=att_out[:],
            )

    nc.compile()
    return nc


def _get_graph():
    global _GRAPH
    if _GRAPH is None:
        _GRAPH = _build_graph()
    return _GRAPH


def make_in_maps(enc, mask, rnn_state, W_rec, w_score):
    import ml_dtypes

    bf = ml_dtypes.bfloat16
    enc_bf = np.asarray(enc, dtype=np.float32).astype(bf)
    # [g, t, core, b, hc, p] view -> per-core [g, p, hc, b, t]
    e6 = enc_bf.reshape(NG, GT, NCORES, BL, HC, 128).transpose(2, 0, 5, 4, 3, 1)
    wrecT = np.ascontiguousarray(
        W_rec.T.astype(np.float32).astype(bf).reshape(RC, 128, H)
    )
    wTh = np.ascontiguousarray(
        w_score.astype(np.float32).astype(bf).reshape(HC, 128).T
    )
    m4 = (np.arange(128)[:, None] % BL == np.arange(128)[None, :] % BL).astype(
        np.float32
    )
    mask = np.asarray(mask, dtype=np.float32)
    rnn = np.asarray(rnn_state, dtype=np.float32)
    in_maps = []
    for c in range(NCORES):
        sl = slice(c * BL, (c + 1) * BL)
        in_maps.append(
            {
                "encT": np.ascontiguousarray(e6[c]),
                "maskd": np.ascontiguousarray(mask[:, sl]),
                "rnnT": np.ascontiguousarray(
                    rnn[sl].T.astype(bf).reshape(RC, 128, BL)
                ),
                "wrecT": wrecT,
                "wT": wTh,
                "m4": m4,
            }
        )
    return in_maps


def kernel(
    encoded_contribution,
    mask,
    rnn_state,
    prev_att_weights,
    W_rec,
    w_score,
    b_score,
):
    from concourse.bass_utils import run_bass_kernel_spmd

    nc = _get_graph()
    in_maps = make_in_maps(
        np.asarray(encoded_contribution),
        np.asarray(mask),
        np.asarray(rnn_state),
        np.asarray(W_rec),
        np.asarray(w_score),
    )
    res = run_bass_kernel_spmd(nc, in_maps, list(range(NCORES)))
    outs = [np.asarray(res.results[c]["out"]) for c in range(NCORES)]
    return np.concatenate([o.T for o in outs], axis=1).astype(np.float32)
